# revision 1
# baseline (speedup 1.0000x reference)
"""Trainium2 Bass kernel for nn_Decoder: LSTM(D=128,H=100) over T=250 + Dense+ReLU.

Strategy
--------
Data-parallel: batch 2048 sharded 8 ways (256/core); tiny weights replicated;
no cross-device communication. x-projection, LSTM recurrence and dense+ReLU
are one fused kernel; the only DRAM traffic is the x preload and the y store.

Everything is feature-major (partition = hidden/gate index, free = batch) so
the recurrent h never needs transposing:

    zT[gate] = Wi[:,gate].T @ xT_t  +  Whb[:,gate].T @ hT_aug   (PSUM accumulate)

x is pre-transposed on the host to [T, D, B_local] fp16 and preloaded entirely
into SBUF (128 KB/partition) at init. The hidden bias bh rides an appended
ones-row in hT (so it accumulates via the matmul); the dense bias bd rides the
per-partition scalar port of the ReLU tensor_scalar op.

The 256-wide local batch is split into two *independent* groups of 128 whose
serial dependency rings (h(t-1) -> matmuls -> sigmoid -> cell update -> tanh
-> h(t)) interleave on the engines, hiding most of the per-step latency.
Per group and step the elementwise work is exactly:

    s    = sigmoid(z[f,g,i,o])        one ScalarE op (g cols pre-scaled x2 on
                                      the host, so s_g = sigmoid(2 z_g))
    v    = s_f * c                    VectorE TT (fp16 2x mode)
    u2   = (s_g - 0.5) * s_i          VectorE scalar_tensor_tensor
                                      (= tanh(z_g)/2 * s_i = (i*g)/2)
    c'   = u2 + v                     VectorE TT; the cell state is kept as
                                      c/2, which makes this a plain add
    tanh = Tanh(c', scale=2)          ScalarE (tanh(2*(c/2)) = tanh(c))
    h    = tanh * s_o                 VectorE TT
    y    = relu(yt + bd)              VectorE tensor_scalar, psum -> sbuf

All transcendentals live in one ACT table set (sigmoid/tanh), loaded once.

PSUM (8 banks): zA/zB x2 parities [100,512] (gate blocks f,g,i,o) = 4 banks,
yA yB dense outputs = 2 banks.
Cell state c and all other intermediates live in SBUF (fp16).

Built as bacc.Bacc so finalize() splits multi-wait instructions into event
semaphores and moves matmul waits onto ldweights (ISA wait-slot limits).
"""

import sys

sys.path.insert(0, "/opt/trn_rl_repo")

from contextlib import ExitStack

import numpy as np

import concourse.bacc as bacc
import concourse.bass as bass
import concourse.tile as tile
from concourse import mybir
from concourse.bass_utils import run_bass_kernel_spmd

B, T, D, H = 2048, 250, 128, 100
NCORES = 8
BL = B // NCORES  # 256 batch per core
GW = BL // 2  # 128 per group

F16 = mybir.dt.float16
F32 = mybir.dt.float32
AF = mybir.ActivationFunctionType
ALU = mybir.AluOpType

# z-tile gate order [f, g, i, o]; (psum_col, weight_col); o emitted first so
# the on-ring sigmoid over cols 0:384 (f,g,i) never waits on the o matmul.
ZF, ZG, ZI, ZO = 0, 128, 256, 384
MM_ORDER = [(ZO, 300), (ZF, 0), (ZG, 100), (ZI, 200)]

YW = 1  # y relu/store batching window (steps)
LAST_RESULTS = None  # test.py reads exec_time_ns / timing off this


def build_program(t_steps=T, bl=BL):
    gw = bl // 2
    # Bacc.finalize() runs the pass pipeline that splits >1-wait
    # instructions into EventSemaphores and moves matmul waits onto
    # ldweights -- walrus rejects the raw Tile output otherwise.
    nc = bacc.Bacc()
    xT_d = nc.dram_tensor("xT", [t_steps, D, bl], F16, kind="ExternalInput")
    wi_d = nc.dram_tensor("Wi", [D, 4 * H], F16, kind="ExternalInput")
    whb_d = nc.dram_tensor("Whb", [H + 1, 4 * H], F16, kind="ExternalInput")
    wd_d = nc.dram_tensor("Wd", [H, H], F16, kind="ExternalInput")
    bd_d = nc.dram_tensor("bd", [H, 1], F32, kind="ExternalInput")
    y_d = nc.dram_tensor("y", [t_steps, H, bl], F32, kind="ExternalOutput")

    with tile.TileContext(nc) as tc, ExitStack() as ctx:
        consts = ctx.enter_context(tc.tile_pool(name="consts", bufs=1))
        hpool = ctx.enter_context(tc.tile_pool(name="hpool", bufs=1))
        zpool = ctx.enter_context(
            tc.tile_pool(name="zpool", bufs=1, space=bass.MemorySpace.PSUM)
        )
        spool = ctx.enter_context(tc.tile_pool(name="spool", bufs=3))
        uvpool = ctx.enter_context(tc.tile_pool(name="uvpool", bufs=3))
        ypool = ctx.enter_context(tc.tile_pool(name="ypool", bufs=5))

        wi_sb = consts.tile([D, 4 * H], F16, name="wi_sb")
        whb_sb = consts.tile([H + 1, 4 * H], F16, name="whb_sb")
        wd_sb = consts.tile([H, H], F16, name="wd_sb")
        bd_sb = consts.tile([H, 1], F32, name="bd_sb")
        nc.sync.dma_start(out=wi_sb[:], in_=wi_d[:])
        nc.sync.dma_start(out=whb_sb[:], in_=whb_d[:])
        nc.sync.dma_start(out=wd_sb[:], in_=wd_d[:])
        nc.sync.dma_start(out=bd_sb[:], in_=bd_d[:])

        # recurrent h, parity-buffered, with the bh ones-row
        hT = {
            g: [hpool.tile([H + 1, gw], F16, name=f"h{g}{p}") for p in range(2)]
            for g in "AB"
        }
        # Engine APs can't start at partition 100, but 96 is legal: write the
        # ones row by memsetting partitions 96:101 to 1.0, then zeroing 0:100.
        # (A DMA'd ones row would put a second sem wait on the h-matmuls,
        # which overflows the Matmult ISA wait slot in walrus.)
        for g in "AB":
            for p in range(2):
                nc.vector.memset(hT[g][p][96 : H + 1, :], 1.0)
        # group A's h(-1)=0 now; group B's is deferred to mid-phase-0 so the
        # two rings start ~half a period out of phase (avoids engine-burst
        # collisions between the rings in steady state)
        nc.vector.memset(hT["A"][1][0:H, :], 0.0)

        zt = {
            g: [
                zpool.tile([H, 512], F32, name=f"z{g}{p}", tag=f"z{g}{p}")
                for p in range(2)
            ]
            for g in "AB"
        }
        # y accumulates YW steps per group in one psum bank; relu+store every YW
        yt = {g: zpool.tile([H, YW * gw], F32, name=f"y{g}", tag=f"y{g}") for g in "AB"}
        # cell state lives in SBUF: cheaper DVE access than PSUM
        ct = {g: hpool.tile([H, 2 * gw], F16, name=f"c{g}") for g in "AB"}
        for g in "AB":
            nc.vector.memset(ct[g][:, gw : 2 * gw], 0.0)  # c(-1) = 0 (parity 1)

        # The whole per-core x fits in SBUF (T*BL*2B = 128 KB/partition):
        # preload it in chunks at init. No per-step DMA, no slot-reuse sems.
        xbig = consts.tile([D, t_steps * bl], F16, name="xbig")
        XCH = 16
        for k in range(0, t_steps, XCH):
            ke = min(k + XCH, t_steps)
            nc.sync.dma_start(
                out=xbig[:, k * bl : ke * bl],
                in_=xT_d[k:ke].rearrange("t d b -> d t b"),
            )
        gcols = {"A": (0, gw), "B": (gw, bl)}

        def x_matmuls(g, t):
            # openers of the z bank for step t (start=True on first toucher)
            x0, x1 = gcols[g]
            first = True
            for pc, wc in MM_ORDER:
                nc.tensor.matmul(
                    zt[g][t % 2][:, pc : pc + gw],
                    wi_sb[:, wc : wc + H],
                    xbig[:, t * bl + x0 : t * bl + x1],
                    start=first,
                    stop=False,
                )
                first = False

        # prologue: step-0 x matmuls
        x_matmuls("A", 0)
        x_matmuls("B", 0)

        y_sb = None

        def phase(g, t):
            nonlocal y_sb
            p, q = t % 2, 1 - (t % 2)
            z = zt[g][p]
            c = ct[g]
            cw, cr = c[:, p * gw : p * gw + gw], c[:, q * gw : q * gw + gw]
            x0, x1 = gcols[g]

            # recurrent matmuls for step t (accumulate onto x contribution)
            last = MM_ORDER[-1][0]
            for pc, wc in MM_ORDER:
                nc.tensor.matmul(
                    z[:, pc : pc + gw],
                    whb_sb[:, wc : wc + H],
                    hT[g][q],
                    start=False,
                    stop=(pc == last),
                )
            if t > 0:
                # dense for step t-1 (after the ring-critical h matmuls)
                sl = (t - 1) % YW
                nc.tensor.matmul(
                    yt[g][:, sl * gw : sl * gw + gw],
                    wd_sb[:],
                    hT[g][q][0:H, :],
                    start=(sl == 0),
                    stop=(sl == YW - 1),
                )
            # x contribution for step t+1 into the other parity bank
            if t + 1 < t_steps:
                x_matmuls(g, t + 1)

            # one sigmoid over all four gate blocks [f,g,i,o]
            s1 = spool.tile([H, 512], F16, name=f"s1{g}{t}", tag=f"s1{g}")
            nc.scalar.activation(s1[:], z[:, 0:512], AF.Sigmoid)
            so = s1[:, ZO : ZO + gw]

            # c' = 2*(s_g - 0.5)*s_i + s_f*c   (fp16 throughout: TT gets 2x)
            v = uvpool.tile([H, gw], F16, name=f"v{g}{t}", tag=f"v{g}")
            nc.vector.tensor_tensor(v[:], cr, s1[:, ZF : ZF + gw], ALU.mult)
            u2 = uvpool.tile([H, gw], F16, name=f"u2{g}{t}", tag=f"u2{g}")
            nc.vector.scalar_tensor_tensor(
                u2[:], s1[:, ZG : ZG + gw], 0.5, s1[:, ZI : ZI + gw],
                ALU.subtract, ALU.mult,
            )
            # state is c/2, so this is a plain add (2x-mode TT on fp16)
            nc.vector.tensor_tensor(cw, u2[:], v[:], ALU.add)
            if t > 0 and (t - 1) % YW == YW - 1:
                # relu(y + bd), emitted after c' so it tends to land in the
                # DVE window where the ring waits on tanh(c)
                ysr = y_sb.rearrange("h (s b) -> h s b", b=bl)
                relu_bi = nc.vector.tensor_scalar(
                    ysr[:, :, x0:x1],
                    yt[g][:].rearrange("h (s b) -> h s b", b=gw),
                    bd_sb[:], 0.0, ALU.add, ALU.max,
                )
                if g == "B":
                    nc.sync.dma_start(
                        out=y_d[t - YW : t].rearrange("s h b -> h s b"),
                        in_=ysr,
                    )
            # h = tanh(c) * s_o = tanh(2 * c/2) * s_o  (free input scale)
            tc_t = uvpool.tile([H, gw], F16, name=f"tc{g}{t}", tag=f"tc{g}")
            nc.scalar.activation(tc_t[:], cw, AF.Tanh, scale=2.0)
            nc.vector.tensor_tensor(hT[g][p][0:H, :], tc_t[:], so, ALU.mult)

        for t in range(t_steps):
            if t > 0 and (t - 1) % YW == YW - 1:
                y_sb = ypool.tile([H, YW * bl], F32, name=f"ysb{t}", tag="ysb")
            phase("A", t)
            if t == 0:
                # deferred: forces group B's ring half a period behind A's
                nc.vector.memset(hT["B"][1][0:H, :], 0.0)
            phase("B", t)

        # epilogue: dense + relu + store for the remaining tail steps
        tl = t_steps - 1
        pl = tl % 2
        tail_n = t_steps % YW or YW  # y-steps still buffered incl. step tl
        sl = tl % YW
        y_sb = ypool.tile([H, tail_n * bl], F32, name="ysb_last", tag="ysb")
        ysr = y_sb.rearrange("h (s b) -> h s b", b=bl)
        for g in "AB":
            x0, x1 = gcols[g]
            nc.tensor.matmul(
                yt[g][:, sl * gw : sl * gw + gw],
                wd_sb[:],
                hT[g][pl][0:H, :],
                start=(sl == 0),
                stop=True,
            )
            nc.vector.tensor_scalar(
                ysr[:, :, x0:x1],
                yt[g][:, 0 : tail_n * gw].rearrange("h (s b) -> h s b", b=gw),
                bd_sb[:], 0.0, ALU.add, ALU.max,
            )
        nc.sync.dma_start(
            out=y_d[t_steps - tail_n : t_steps].rearrange("s h b -> h s b"),
            in_=ysr,
        )

    nc.finalize()
    return nc


def prep_inputs(x, Wi, Wh, bh, Wd, bd):
    """Host-side prep: shard + transpose x, reorder gates to [f,g,i,o],
    fold bh into an extra Wh row, pre-scale for the all-sigmoid scheme,
    cast matmul operands to fp16."""
    idx = np.r_[100:200, 200:300, 0:100, 300:400]  # [f, g, i, o]
    bf = np.float16
    wi_r = np.ascontiguousarray(Wi[:, idx]).astype(np.float32)
    whb = np.concatenate([Wh[:, idx], bh[idx][None, :]], axis=0).astype(np.float32)
    # g gate feeds sigmoid(2*z_g): double its columns (incl. bias)
    wi_r[:, 100:200] *= 2.0
    whb[:, 100:200] *= 2.0
    wd_b = np.ascontiguousarray(Wd).astype(bf)
    wi_r = wi_r.astype(bf)
    whb = whb.astype(bf)
    bd_c = np.ascontiguousarray(bd.reshape(H, 1).astype(np.float32))
    t_steps = x.shape[1]
    xs = x.reshape(NCORES, BL, t_steps, D).transpose(0, 2, 3, 1)  # [8, T, D, BL]
    in_maps = []
    for c in range(NCORES):
        in_maps.append(
            {
                "xT": np.ascontiguousarray(xs[c]).astype(bf),
                "Wi": wi_r,
                "Whb": whb,
                "Wd": wd_b,
                "bd": bd_c,
            }
        )
    return in_maps


def kernel(x, Wi, Wh, bh, Wd, bd):
    global LAST_RESULTS
    nc = build_program()
    in_maps = prep_inputs(x, Wi, Wh, bh, Wd, bd)
    res = run_bass_kernel_spmd(nc, in_maps, list(range(NCORES)))
    LAST_RESULTS = res
    outs = [res.results[c]["y"].transpose(2, 0, 1) for c in range(NCORES)]
    return np.ascontiguousarray(np.concatenate(outs, axis=0), dtype=np.float32)

